# revision 26
# baseline (speedup 1.0000x reference)
"""Causal multi-head self-attention with RoPE on 8 Trainium2 NeuronCores.

Sharding: data parallel over batch (2) x tensor parallel over heads (4 groups
of 4 heads).  Core c handles batch b = c // 4, head group hg = c % 4.

Per-core dataflow (everything stays in "transposed" [feature, seq] layouts so
no on-device transposes are ever needed; all matmuls fp16 with fp32 psum):
  QT = wqT.T @ xT   [256, 2048]   (d-contraction on partitions)
  RoPE via a constant shuffle matmul: rot = QT*cosT + (S @ (QT*sinT))
  V  = xT.T @ wvT   [2048, 256] -> fp16, augmented with a ones column per head
  per head h:
    scores^T[ktile j] = Krot_h[:,128j:128j+128].T @ Qrot_h   (k on partitions)
    exp over causal-PACKED psum pieces of 1024 cols that span k-tile
    boundaries (score matmuls split at 512-col psum-bank boundaries, one
    accumulation group per bank); diagonal 128x128 blocks *= triangular mask
    per 512-query chunk: psum[65,512] = sum_j V_aug_j.T @ expP_j  (fp16)
       row 64 is the softmax denominator (ones column of V_aug)
    A^T = psum[:64] * bcast(1/psum[64])  -> fp16
  outT_partial = woT.T @ A^T  [1024, 2048]  (host sums the 4 partials per b)

Schedule notes (what the ~190us figure depends on):
  - input DMA is split across the SP + ACT hardware DGE queues (and SWDGE
    for late bulk), ordered by first use: the first projection group starts
    right after the runtime preamble and the PE p-state ramps once.
  - score pieces clock the stream; after each piece's exp, pending work
    (PV slices of the previous head, softmax finalize, outproj blocks) is
    emitted so the in-order PE queue is never head-of-line blocked by a
    piece that still waits on exp.
  - outproj for query chunk c runs mid-stream during head 3 (deferred until
    the flush cursor passes chunk c's tri-mask) so output DMA never bunches
    at the end; output casts alternate DVE/ACT, output DMA round-robins
    over the three trigger queues.
Host: out[b] += outT_partial.T per core.
"""

import numpy as np

import concourse.bass as bass
import concourse.mybir as mybir
import concourse.tile as tile
from concourse import bacc
from concourse.bass_utils import run_bass_kernel_spmd

F32 = mybir.dt.float32
F32R = mybir.dt.float32r
F16 = mybir.dt.float16

B, S, D, H, DH = 2, 2048, 1024, 16, 64
ROPE_THETA = 10000.0
NCORE = 8
HPG = 4          # heads per group (per core)
P = 128
NKT = S // P     # 16 k-tiles
NQC = S // 512   # 4 query chunks

# expP storage: k-tile j's columns start at global q = 512*(j//4); width below.
_W = [S - P * j for j in range(NKT)]
_OFF = np.concatenate([[0], np.cumsum(_W)]).astype(int)
EXP_TOT = int(_OFF[-1])  # 17408 columns of fp16 -> 34KB/partition


def build_program():
    nc = bacc.Bacc(
        "TRN2", target_bir_lowering=False, debug=False, num_devices=NCORE
    )

    xT = nc.dram_tensor("xT", [D, S], F16, kind="ExternalInput")
    wqT = nc.dram_tensor("wqT", [D, 256], F16, kind="ExternalInput")
    wkT = nc.dram_tensor("wkT", [D, 256], F16, kind="ExternalInput")
    wvT = nc.dram_tensor("wvT", [D, 256], F16, kind="ExternalInput")
    woT = nc.dram_tensor("woT", [256, D], F16, kind="ExternalInput")
    cosT = nc.dram_tensor("cosT", [P, S], F16, kind="ExternalInput")
    sinT = nc.dram_tensor("sinT", [P, S], F16, kind="ExternalInput")
    ST = nc.dram_tensor("ST", [P, P], F16, kind="ExternalInput")
    trimask = nc.dram_tensor("trimask", [P, P], F16, kind="ExternalInput")

    outT = nc.dram_tensor("outT", [D, S], F16, kind="ExternalOutput")

    with tile.TileContext(nc) as tc:
        with (
            tc.tile_pool(name="const", bufs=1) as cpool,
            tc.tile_pool(name="qkv", bufs=1) as qkv,
            tc.tile_pool(name="psum", bufs=1, space="PSUM") as psum,
            tc.tile_pool(name="agp", bufs=4) as agp,
        ):
            tri_sb = cpool.tile([P, P], F16, tag="tri")
            wo_sb = cpool.tile([P, 2, D], F16, tag="wo")
            qrot = qkv.tile([P, 2, S], F16, tag="qrot")
            krot = qkv.tile([P, 2, S], F16, tag="krot")
            v_sb = qkv.tile([P, NKT, HPG, DH + 1], F16, tag="v")
            at_sb = qkv.tile([P, 2, S], F16, tag="at")

            # ---------------- phase 1: projections + rope -----------------
            with tc.tile_pool(name="p1", bufs=1) as p1:
                wq_sb = p1.tile([P, 8, 256], F16, tag="wq")
                xt_sb = p1.tile([P, 8, S], F16, tag="xt")
                wk_sb = p1.tile([P, 8, 256], F16, tag="wk")
                wv_sb = p1.tile([P, 8, 256], F16, tag="wv")
                cos_sb = p1.tile([P, S], F16, tag="cos")
                sin_sb = p1.tile([P, S], F16, tag="sin")
                st_sb = p1.tile([P, P], F16, tag="st")

                # Fine-grained DMA ordered by first use, split across BOTH
                # hardware DGE queues (sync=SP + scalar=ACT; scalar is idle
                # until the first exp) so the first projection group can start
                # right after the runtime preamble.
                wqr = wqT.rearrange("(n p) m -> p n m", p=P)
                wkr = wkT.rearrange("(n p) m -> p n m", p=P)
                wvr = wvT.rearrange("(n p) m -> p n m", p=P)
                xr = xT.rearrange("(n p) m -> p n m", p=P)
                # sync+scalar carry the urgent stream symmetrically (scalar
                # is free until the first exp ~25us); gpsimd (SWDGE, shares
                # the Pool engine with the warm-up) only gets late bulk.
                nc.sync.dma_start(out=wq_sb[:, :, 0:P], in_=wqr[:, :, 0:P])
                nc.scalar.dma_start(
                    out=xt_sb[:, 4:8, 0:512], in_=xr[:, 4:8, 0:512]
                )
                nc.sync.dma_start(out=xt_sb[:, 0:4, 0:512], in_=xr[:, 0:4, 0:512])
                nc.scalar.dma_start(
                    out=xt_sb[:, 4:8, 512:1024], in_=xr[:, 4:8, 512:1024]
                )
                nc.sync.dma_start(
                    out=xt_sb[:, 0:4, 512:1024], in_=xr[:, 0:4, 512:1024]
                )
                nc.gpsimd.dma_start(out=sin_sb[:], in_=sinT[:, :])
                nc.gpsimd.dma_start(out=cos_sb[:], in_=cosT[:, :])
                nc.gpsimd.dma_start(out=st_sb[:], in_=ST[:, :])
                nc.scalar.dma_start(out=wq_sb[:, :, P:256], in_=wqr[:, :, P:256])
                nc.sync.dma_start(out=wk_sb[:, :, 0:P], in_=wkr[:, :, 0:P])
                for sc in range(2, 4):
                    ssl = bass.ts(sc, 512)
                    nc.sync.dma_start(out=xt_sb[:, 0:4, ssl], in_=xr[:, 0:4, ssl])
                    nc.scalar.dma_start(out=xt_sb[:, 4:8, ssl], in_=xr[:, 4:8, ssl])
                nc.scalar.dma_start(out=wk_sb[:, :, P:256], in_=wkr[:, :, P:256])
                nc.gpsimd.dma_start(out=tri_sb[:], in_=trimask[:, :])

                nc.vector.memset(v_sb[:, :, :, DH:DH + 1], 1.0)

                # preload the gpsimd custom-op library (first use pays ~7us)
                warm_a = p1.tile([1, 64], F32, tag="warm_a")
                warm_b = p1.tile([64, 64], F32, tag="warm_b")
                nc.vector.memset(warm_a[:], 1.0)
                nc.gpsimd.partition_broadcast(warm_b[:], warm_a[:])

                nc.gpsimd.dma_start(out=wv_sb[:], in_=wvr)
                nc.gpsimd.dma_start(
                    out=wo_sb[:], in_=woT.rearrange("(n p) m -> p n m", p=P)
                )

                # preload the Exp activation table (first use pays ~1.3us)
                warm_e = p1.tile([1, 8], F16, tag="warm_e")
                nc.scalar.activation(
                    out=warm_e[:], in_=warm_a[0:1, 0:8],
                    func=mybir.ActivationFunctionType.Exp, scale=1.0,
                )


                # Q/K projections + rope
                for mt in range(2):
                    for w_sb, rot in ((wq_sb, qrot), (wk_sb, krot)):
                        for sc in range(4):
                            ssl = bass.ts(sc, 512)
                            pp = psum.tile([P, 512], F32, tag="proj", bufs=3)
                            for dt in range(8):
                                nc.tensor.matmul(
                                    pp[:],
                                    w_sb[:, dt, P * mt:P * (mt + 1)],
                                    xt_sb[:, dt, ssl],
                                    start=(dt == 0),
                                    stop=(dt == 7),
                                )
                            t_s = p1.tile([P, 512], F16, tag="ts", bufs=3)
                            nc.vector.tensor_tensor(
                                out=t_s[:], in0=pp[:], in1=sin_sb[:, ssl],
                                op=mybir.AluOpType.mult,
                            )
                            sh = psum.tile([P, 512], F32, tag="shuf", bufs=1)
                            nc.tensor.matmul(
                                sh[:], st_sb[:], t_s[:], start=True, stop=True
                            )
                            nc.vector.tensor_tensor(
                                out=rot[:, mt, ssl], in0=pp[:],
                                in1=cos_sb[:, ssl], op=mybir.AluOpType.mult,
                            )
                            nc.vector.tensor_tensor(
                                out=rot[:, mt, ssl], in0=rot[:, mt, ssl],
                                in1=sh[:], op=mybir.AluOpType.add,
                            )

                # V projection -> fp16 V_aug (dense PE bridge into attention)
                for st in range(NKT):
                    vp = psum.tile([P, 256], F32, tag="shuf", bufs=1, name="vp")
                    for dt in range(8):
                        nc.tensor.matmul(
                            vp[:],
                            xt_sb[:, dt, P * st:P * (st + 1)],
                            wv_sb[:, dt, :],
                            start=(dt == 0),
                            stop=(dt == 7),
                        )
                    nc.vector.tensor_copy(
                        out=v_sb[:, st, :, 0:DH],
                        in_=vp.rearrange("p (h d) -> p h d", h=HPG),
                    )

            # ---------------- phase 2+3: attention + output projection -----
            with tc.tile_pool(name="atmp", bufs=3) as atmp:
                eps = [
                    qkv.tile([P, EXP_TOT], F16, tag=f"expp{i}", name=f"ep{i}")
                    for i in range(3)
                ]
                piece_ctr = [0]

                # Pending fine-grained PE work (PV matmul slices, softmax
                # finalize chains, outproj blocks).  One item is emitted after
                # every score piece so the PE instruction stream always has
                # ready work queued ahead of a piece that waits on exp — the
                # PE sequencer is in-order, so a blocked score piece would
                # otherwise head-of-line-block everything behind it.
                pending = []

                def filler(n=1):
                    for _ in range(n):
                        if not pending:
                            return
                        pending.pop(0)()

                # Scores psum pieces are PIECE_W packed columns wide and span
                # k-tile boundaries, so one exp instruction covers ~1536
                # columns — a third of the per-instruction ACT overhead of
                # the per-tile layout.
                PIECE_W = 1024
                sstt = {"cur": None, "p": -1, "tris": [], "deferred": []}

                def flush_piece(h):
                    if sstt["cur"] is None:
                        return
                    ep = eps[h % 3]
                    p = sstt["p"]
                    base = PIECE_W * p
                    wid = min(PIECE_W, EXP_TOT - base)
                    nc.scalar.activation(
                        out=ep[:, base:base + wid],
                        in_=sstt["cur"][:, 0:wid],
                        func=mybir.ActivationFunctionType.Exp,
                        scale=0.125,
                    )
                    # triangular masks for diagonal blocks inside this piece
                    for toff in sstt["tris"]:
                        nc.vector.tensor_tensor(
                            out=ep[:, toff:toff + P], in0=ep[:, toff:toff + P],
                            in1=tri_sb[:], op=mybir.AluOpType.mult,
                        )
                    sstt["tris"] = []
                    sstt["cur"] = None
                    # release work that was waiting for masks up to this col
                    done = base + wid
                    rest = []
                    for thr, fn in sstt["deferred"]:
                        if thr <= done:
                            fn()
                        else:
                            rest.append((thr, fn))
                    sstt["deferred"] = rest
                    filler(3)

                def emit_scores(h, j):
                    th, bs = h // 2, 64 * (h % 2)
                    qh = qrot[bs:bs + 64, th, :]
                    kh = krot[bs:bs + 64, th, :]
                    c0, r = j // 4, j % 4
                    off = int(_OFF[j])
                    for c in range(c0, 4):
                        q0 = 512 * c + (128 * r if c == c0 else 0)
                        q1 = 512 * (c + 1)
                        a = off + q0 - 128 * j
                        while q0 < q1:
                            p = a // PIECE_W
                            if p != sstt["p"] or sstt["cur"] is None:
                                flush_piece(h)
                                sstt["p"] = p
                                tag = ("sA", "sB")[piece_ctr[0] % 2]
                                piece_ctr[0] += 1
                                sstt["cur"] = psum.tile(
                                    [P, PIECE_W], F32, tag=tag, bufs=1,
                                    name="sp",
                                )
                            # split at 512-col (psum bank / zero-region)
                            # boundaries; the packed fill is gapless left to
                            # right, so a write starting at a bank boundary
                            # opens that bank's accumulation group and a
                            # write ending at one closes it.
                            take = min(q1 - q0, 512 - a % 512)
                            nc.tensor.matmul(
                                sstt["cur"][:, a - PIECE_W * p:
                                            a - PIECE_W * p + take],
                                kh[:, P * j:P * (j + 1)],
                                qh[:, q0:q0 + take],
                                start=(a % 512 == 0),
                                stop=((a + take) % 512 == 0),
                                skip_group_check=True,
                            )
                            q0 += take
                            a += take
                    sstt["tris"].append(off)

                def queue_pv(h, c):
                    ep = eps[h % 3]
                    th, bs = h // 2, 64 * (h % 2)
                    state = {}
                    last_j = 4 * c + 3

                    def mk_slice(j0):
                        def emit():
                            if "pv" not in state:
                                state["pv"] = psum.tile(
                                    [P, 512], F32, tag="proj", bufs=3,
                                    name="pv",
                                )
                            pv = state["pv"]
                            for j in range(j0, min(j0 + 3, last_j + 1)):
                                off = int(_OFF[j])
                                if j // 4 == c:
                                    rr = j % 4
                                    n = 512 - 128 * rr
                                    nc.tensor.matmul(
                                        pv[0:DH + 1, 128 * rr:512],
                                        v_sb[:, j, h, :],
                                        ep[:, off:off + n],
                                        start=(j == 0), stop=(j == last_j),
                                    )
                                else:
                                    st_col = off + 512 * c - 128 * j
                                    nc.tensor.matmul(
                                        pv[0:DH + 1, :],
                                        v_sb[:, j, h, :],
                                        ep[:, st_col:st_col + 512],
                                        start=(j == 0), stop=(j == last_j),
                                    )
                        return emit

                    def finalize():
                        pv = state["pv"]
                        den = atmp.tile([1, 512], F32, tag="den")
                        nc.vector.tensor_copy(out=den[:], in_=pv[DH:DH + 1, :])
                        recip = atmp.tile([1, 512], F32, tag="recip")
                        nc.vector.reciprocal_approx_fast(
                            out=recip[:], in_=den[:]
                        )
                        bcast = atmp.tile([64, 512], F32, tag="bcast")
                        nc.gpsimd.partition_broadcast(bcast[:], recip[:])
                        nc.vector.tensor_tensor(
                            out=at_sb[bs:bs + 64, th, 512 * c:512 * (c + 1)],
                            in0=pv[0:DH, :], in1=bcast[:],
                            op=mybir.AluOpType.mult,
                        )

                    for j0 in range(0, last_j + 1, 3):
                        pending.append(mk_slice(j0))
                    pending.append(finalize)

                def queue_outproj(sc):
                    # outT_partial[:, sc] = sum over the 256 LOCAL attention
                    # dims (this core's 4 heads); host sums the partials
                    ssl = bass.ts(sc, 512)

                    def mk_block(ot):
                        def emit():
                            osl = bass.ts(ot, P)
                            if ot % 2 == 0:
                                po = psum.tile(
                                    [P, 512], F32, tag="proj", bufs=3,
                                    name="po",
                                )
                            else:
                                po = psum.tile(
                                    [P, 512], F32, tag="shuf", bufs=1,
                                    name="po",
                                )
                            for ct in range(2):
                                nc.tensor.matmul(
                                    po[:, 0:512],
                                    wo_sb[:, ct, osl],
                                    at_sb[:, ct, ssl],
                                    start=(ct == 0), stop=(ct == 1),
                                )
                            ob = agp.tile([P, 512], F16, tag="ob", name="ob")
                            if ot % 2 == 0:
                                nc.vector.tensor_copy(
                                    out=ob[:], in_=po[:, 0:512]
                                )
                            else:
                                nc.scalar.activation(
                                    out=ob[:], in_=po[:, 0:512],
                                    func=mybir.ActivationFunctionType.Copy,
                                    scale=1.0,
                                )
                            oq = (nc.sync, nc.gpsimd, nc.scalar)[ot % 3]
                            oq.dma_start(
                                out=outT[P * ot:P * (ot + 1), ssl], in_=ob[:]
                            )
                        return emit

                    for ot in range(8):
                        pending.append(mk_block(ot))

                # software pipeline: score pieces of head h clock the stream;
                # after each piece one pending item (a PV slice of head h-1,
                # a softmax finalize, or an outproj block) is emitted so the
                # PE always has ready work directly behind a piece that still
                # waits on exp.
                for h in range(HPG):
                    for j in range(NKT):
                        emit_scores(h, j)
                        if h >= 1 and j % 4 == 1:
                            queue_pv(h - 1, j // 4)
                        if h == 3 and j % 4 == 3:
                            c = j // 4
                            if c < 3:
                                # head-3 PV must not enter the stream until
                                # the tri-mask of its last tile has been
                                # emitted (at the flush covering that block)
                                thr = int(_OFF[j]) + P

                                def mk_pv(cc):
                                    def fn():
                                        queue_pv(3, cc)
                                    return fn

                                def mk_op(cc):
                                    def fn():
                                        queue_outproj(cc)
                                    return fn

                                sstt["deferred"].append((thr, mk_pv(c)))
                                sstt["deferred"].append(
                                    (min(thr + 2048, EXP_TOT), mk_op(c))
                                )
                    flush_piece(h)
                queue_pv(3, 3)
                queue_outproj(3)
                while pending:
                    pending.pop(0)()

    nc.compile()
    return nc


_PROGRAM = None


def _get_program():
    global _PROGRAM
    if _PROGRAM is None:
        _PROGRAM = build_program()
    return _PROGRAM


def _host_consts(token_positions):
    pos = np.asarray(token_positions, dtype=np.float32)
    inv = (
        ROPE_THETA ** (-np.arange(0, DH, 2, dtype=np.float32) / DH)
    ).astype(np.float32)
    ang = pos[:, None] * inv[None, :]  # [S, 32]
    cos, sin = np.cos(ang), np.sin(ang)
    rows = (np.arange(P) % DH) // 2
    cosT = np.ascontiguousarray(cos.T[rows]).astype(np.float16)
    sinT = np.ascontiguousarray(sin.T[rows]).astype(np.float16)
    Smat = np.zeros((P, P), dtype=np.float32)
    idx = np.arange(0, P, 2)
    Smat[idx, idx + 1] = -1.0
    Smat[idx + 1, idx] = 1.0
    ST = np.ascontiguousarray(Smat.T).astype(np.float16)
    tri = (np.arange(P)[None, :] >= np.arange(P)[:, None]).astype(np.float16)
    return cosT, sinT, ST, tri


def _make_in_maps(x, W_q, W_k, W_v, W_o, token_positions):
    cosT, sinT, ST, tri = _host_consts(token_positions)
    x = np.asarray(x, dtype=np.float32)
    maps = []
    for core in range(NCORE):
        b, hg = core // 4, core % 4
        hsl = slice(256 * hg, 256 * (hg + 1))
        # W_o columns for this core's local attention dims (its 4 heads);
        # each core emits a full [1024, 2048] partial that the host sums.
        wo_p = np.asarray(W_o, dtype=np.float32)[:, hsl].T   # [256 c, 1024 o]
        maps.append(
            {
                "xT": np.ascontiguousarray(x[b].T).astype(np.float16),
                "wqT": np.ascontiguousarray(np.asarray(W_q, np.float32)[hsl].T).astype(np.float16),
                "wkT": np.ascontiguousarray(np.asarray(W_k, np.float32)[hsl].T).astype(np.float16),
                "wvT": np.ascontiguousarray(np.asarray(W_v, np.float32)[hsl].T).astype(np.float16),
                "woT": np.ascontiguousarray(wo_p).astype(np.float16),
                "cosT": cosT,
                "sinT": sinT,
                "ST": ST,
                "trimask": tri,
            }
        )
    return maps


def _assemble(results):
    out = np.zeros((B, S, D), dtype=np.float32)
    for core in range(NCORE):
        b = core // 4
        out[b] += results[core]["outT"].astype(np.float32).T
    return out


def _run(in_maps, trace=False):
    nc = _get_program()
    tmpdir = None
    if trace:
        import tempfile

        tmpdir = tempfile.mkdtemp(prefix="ntff_", dir="/tmp")
    res = run_bass_kernel_spmd(
        nc, in_maps, list(range(NCORE)), trace=trace, tmpdir=tmpdir
    )
    return res


def kernel(x, W_q, W_k, W_v, W_o, token_positions):
    in_maps = _make_in_maps(x, W_q, W_k, W_v, W_o, token_positions)
    res = _run(in_maps)
    return _assemble(res.results)


def _install_profile_hook():
    """The agent image's antenv lacks axon_hooks; shim it so trace=True works."""
    import sys
    import types

    try:
        from antenv.axon_hooks import get_axon_ntff_profile_hook  # noqa: F401
        return
    except ImportError:
        pass
    import antenv
    from trn_agent_boot.trn_boot import _ntff_profile_via_ctypes

    mod = types.ModuleType("antenv.axon_hooks")
    _hook = {"h": None}
    mod.set_axon_ntff_profile_hook = lambda h: _hook.__setitem__("h", h)
    mod.get_axon_ntff_profile_hook = lambda: _hook["h"]
    sys.modules["antenv.axon_hooks"] = mod
    antenv.axon_hooks = mod
    mod.set_axon_ntff_profile_hook(
        _ntff_profile_via_ctypes("/opt/axon/libaxon_pjrt.so")
    )
    import concourse.bass_utils as bu

    bu.upload_artifacts = lambda d: f"file://{d}"


def kernel_traced(x, W_q, W_k, W_v, W_o, token_positions):
    """Returns (output, exec_time_ns, trace_path)."""
    _install_profile_hook()
    in_maps = _make_in_maps(x, W_q, W_k, W_v, W_o, token_positions)
    res = _run(in_maps, trace=True)
    trace_path = None
    if res.instructions_and_trace is not None:
        trace_path = res.instructions_and_trace[1]
    return _assemble(res.results), res.exec_time_ns, trace_path



# revision 27
# speedup vs baseline: 1.0029x; 1.0029x over previous
"""Causal multi-head self-attention with RoPE on 8 Trainium2 NeuronCores.

Sharding: data parallel over batch (2) x tensor parallel over heads (4 groups
of 4 heads).  Core c handles batch b = c // 4, head group hg = c % 4.

Per-core dataflow (everything stays in "transposed" [feature, seq] layouts so
no on-device transposes are ever needed; all matmuls fp16 with fp32 psum):
  QT = wqT.T @ xT   [256, 2048]   (d-contraction on partitions)
  RoPE via a constant shuffle matmul: rot = QT*cosT + (S @ (QT*sinT))
  V  = xT.T @ wvT   [2048, 256] -> fp16, augmented with a ones column per head
  per head h:
    scores^T[ktile j] = Krot_h[:,128j:128j+128].T @ Qrot_h   (k on partitions)
    exp over causal-PACKED psum pieces of 1024 cols that span k-tile
    boundaries (score matmuls split at 512-col psum-bank boundaries, one
    accumulation group per bank); diagonal 128x128 blocks *= triangular mask
    per 512-query chunk: psum[65,512] = sum_j V_aug_j.T @ expP_j  (fp16)
       row 64 is the softmax denominator (ones column of V_aug)
    A^T = psum[:64] * bcast(1/psum[64])  -> fp16
  outT_partial = woT.T @ A^T  [1024, 2048]  (host sums the 4 partials per b)

Schedule notes (what the ~190us figure depends on):
  - input DMA is split across the SP + ACT hardware DGE queues (and SWDGE
    for late bulk), ordered by first use: the first projection group starts
    right after the runtime preamble and the PE p-state ramps once.
  - score pieces clock the stream; after each piece's exp, pending work
    (PV slices of the previous head, softmax finalize, outproj blocks) is
    emitted so the in-order PE queue is never head-of-line blocked by a
    piece that still waits on exp.
  - outproj for query chunk c runs mid-stream during head 3 (deferred until
    the flush cursor passes chunk c's tri-mask) so output DMA never bunches
    at the end; output casts alternate DVE/ACT, output DMA round-robins
    over the three trigger queues.
Host: out[b] += outT_partial.T per core.
"""

import numpy as np

import concourse.bass as bass
import concourse.mybir as mybir
import concourse.tile as tile
from concourse import bacc
from concourse.bass_utils import run_bass_kernel_spmd

F32 = mybir.dt.float32
F32R = mybir.dt.float32r
F16 = mybir.dt.float16

B, S, D, H, DH = 2, 2048, 1024, 16, 64
ROPE_THETA = 10000.0
NCORE = 8
HPG = 4          # heads per group (per core)
P = 128
NKT = S // P     # 16 k-tiles
NQC = S // 512   # 4 query chunks

# expP storage: k-tile j's columns start at global q = 512*(j//4); width below.
_W = [S - P * j for j in range(NKT)]
_OFF = np.concatenate([[0], np.cumsum(_W)]).astype(int)
EXP_TOT = int(_OFF[-1])  # 17408 columns of fp16 -> 34KB/partition


def build_program():
    nc = bacc.Bacc(
        "TRN2", target_bir_lowering=False, debug=False, num_devices=NCORE
    )

    xT = nc.dram_tensor("xT", [D, S], F16, kind="ExternalInput")
    wqT = nc.dram_tensor("wqT", [D, 256], F16, kind="ExternalInput")
    wkT = nc.dram_tensor("wkT", [D, 256], F16, kind="ExternalInput")
    wvT = nc.dram_tensor("wvT", [D, 256], F16, kind="ExternalInput")
    woT = nc.dram_tensor("woT", [256, D], F16, kind="ExternalInput")
    cosT = nc.dram_tensor("cosT", [P, S], F16, kind="ExternalInput")
    sinT = nc.dram_tensor("sinT", [P, S], F16, kind="ExternalInput")
    ST = nc.dram_tensor("ST", [P, P], F16, kind="ExternalInput")
    trimask = nc.dram_tensor("trimask", [P, P], F16, kind="ExternalInput")

    outT = nc.dram_tensor("outT", [D, S], F16, kind="ExternalOutput")

    with tile.TileContext(nc) as tc:
        with (
            tc.tile_pool(name="const", bufs=1) as cpool,
            tc.tile_pool(name="qkv", bufs=1) as qkv,
            tc.tile_pool(name="psum", bufs=1, space="PSUM") as psum,
            tc.tile_pool(name="agp", bufs=4) as agp,
        ):
            tri_sb = cpool.tile([P, P], F16, tag="tri")
            wo_sb = cpool.tile([P, 2, D], F16, tag="wo")
            qrot = qkv.tile([P, 2, S], F16, tag="qrot")
            krot = qkv.tile([P, 2, S], F16, tag="krot")
            v_sb = qkv.tile([P, NKT, HPG, DH + 1], F16, tag="v")
            at_sb = qkv.tile([P, 2, S], F16, tag="at")

            # ---------------- phase 1: projections + rope -----------------
            with tc.tile_pool(name="p1", bufs=1) as p1:
                wq_sb = p1.tile([P, 8, 256], F16, tag="wq")
                xt_sb = p1.tile([P, 8, S], F16, tag="xt")
                wk_sb = p1.tile([P, 8, 256], F16, tag="wk")
                wv_sb = p1.tile([P, 8, 256], F16, tag="wv")
                cos_sb = p1.tile([P, S], F16, tag="cos")
                sin_sb = p1.tile([P, S], F16, tag="sin")
                st_sb = p1.tile([P, P], F16, tag="st")

                # Fine-grained DMA ordered by first use, split across BOTH
                # hardware DGE queues (sync=SP + scalar=ACT; scalar is idle
                # until the first exp) so the first projection group can start
                # right after the runtime preamble.
                wqr = wqT.rearrange("(n p) m -> p n m", p=P)
                wkr = wkT.rearrange("(n p) m -> p n m", p=P)
                wvr = wvT.rearrange("(n p) m -> p n m", p=P)
                xr = xT.rearrange("(n p) m -> p n m", p=P)
                # sync+scalar carry the urgent stream symmetrically (scalar
                # is free until the first exp ~25us); gpsimd (SWDGE, shares
                # the Pool engine with the warm-up) only gets late bulk.
                nc.sync.dma_start(out=wq_sb[:, :, 0:P], in_=wqr[:, :, 0:P])
                nc.scalar.dma_start(
                    out=xt_sb[:, 4:8, 0:512], in_=xr[:, 4:8, 0:512]
                )
                nc.sync.dma_start(out=xt_sb[:, 0:4, 0:512], in_=xr[:, 0:4, 0:512])
                nc.scalar.dma_start(
                    out=xt_sb[:, 4:8, 512:1024], in_=xr[:, 4:8, 512:1024]
                )
                nc.sync.dma_start(
                    out=xt_sb[:, 0:4, 512:1024], in_=xr[:, 0:4, 512:1024]
                )
                nc.gpsimd.dma_start(out=sin_sb[:], in_=sinT[:, :])
                nc.gpsimd.dma_start(out=cos_sb[:], in_=cosT[:, :])
                nc.gpsimd.dma_start(out=st_sb[:], in_=ST[:, :])
                nc.scalar.dma_start(out=wq_sb[:, :, P:256], in_=wqr[:, :, P:256])
                nc.sync.dma_start(out=wk_sb[:, :, 0:P], in_=wkr[:, :, 0:P])
                for sc in range(2, 4):
                    ssl = bass.ts(sc, 512)
                    nc.sync.dma_start(out=xt_sb[:, 0:4, ssl], in_=xr[:, 0:4, ssl])
                    nc.scalar.dma_start(out=xt_sb[:, 4:8, ssl], in_=xr[:, 4:8, ssl])
                nc.scalar.dma_start(out=wk_sb[:, :, P:256], in_=wkr[:, :, P:256])
                nc.gpsimd.dma_start(out=tri_sb[:], in_=trimask[:, :])

                nc.vector.memset(v_sb[:, :, :, DH:DH + 1], 1.0)

                # preload the gpsimd custom-op library (first use pays ~7us)
                warm_a = p1.tile([1, 64], F32, tag="warm_a")
                warm_b = p1.tile([64, 64], F32, tag="warm_b")
                nc.vector.memset(warm_a[:], 1.0)
                nc.gpsimd.partition_broadcast(warm_b[:], warm_a[:])

                nc.gpsimd.dma_start(out=wv_sb[:], in_=wvr)
                nc.gpsimd.dma_start(
                    out=wo_sb[:], in_=woT.rearrange("(n p) m -> p n m", p=P)
                )

                # preload the Exp activation table (first use pays ~1.3us)
                warm_e = p1.tile([1, 8], F16, tag="warm_e")
                nc.scalar.activation(
                    out=warm_e[:], in_=warm_a[0:1, 0:8],
                    func=mybir.ActivationFunctionType.Exp, scale=1.0,
                )


                # Q/K projections + rope
                for mt in range(2):
                    for w_sb, rot in ((wq_sb, qrot), (wk_sb, krot)):
                        for sc in range(4):
                            ssl = bass.ts(sc, 512)
                            pp = psum.tile([P, 512], F32, tag="proj", bufs=3)
                            for dt in range(8):
                                nc.tensor.matmul(
                                    pp[:],
                                    w_sb[:, dt, P * mt:P * (mt + 1)],
                                    xt_sb[:, dt, ssl],
                                    start=(dt == 0),
                                    stop=(dt == 7),
                                )
                            t_s = p1.tile([P, 512], F16, tag="ts", bufs=3)
                            nc.vector.tensor_tensor(
                                out=t_s[:], in0=pp[:], in1=sin_sb[:, ssl],
                                op=mybir.AluOpType.mult,
                            )
                            sh = psum.tile([P, 512], F32, tag="shuf", bufs=1)
                            nc.tensor.matmul(
                                sh[:], st_sb[:], t_s[:], start=True, stop=True
                            )
                            nc.vector.tensor_tensor(
                                out=rot[:, mt, ssl], in0=pp[:],
                                in1=cos_sb[:, ssl], op=mybir.AluOpType.mult,
                            )
                            nc.vector.tensor_tensor(
                                out=rot[:, mt, ssl], in0=rot[:, mt, ssl],
                                in1=sh[:], op=mybir.AluOpType.add,
                            )

                # V projection -> fp16 V_aug (dense PE bridge into attention)
                for st in range(NKT):
                    vp = psum.tile([P, 256], F32, tag="shuf", bufs=1, name="vp")
                    for dt in range(8):
                        nc.tensor.matmul(
                            vp[:],
                            xt_sb[:, dt, P * st:P * (st + 1)],
                            wv_sb[:, dt, :],
                            start=(dt == 0),
                            stop=(dt == 7),
                        )
                    nc.vector.tensor_copy(
                        out=v_sb[:, st, :, 0:DH],
                        in_=vp.rearrange("p (h d) -> p h d", h=HPG),
                    )

            # ---------------- phase 2+3: attention + output projection -----
            with tc.tile_pool(name="atmp", bufs=3) as atmp:
                eps = [
                    qkv.tile([P, EXP_TOT], F16, tag=f"expp{i}", name=f"ep{i}")
                    for i in range(3)
                ]
                piece_ctr = [0]

                # Pending fine-grained PE work (PV matmul slices, softmax
                # finalize chains, outproj blocks).  One item is emitted after
                # every score piece so the PE instruction stream always has
                # ready work queued ahead of a piece that waits on exp — the
                # PE sequencer is in-order, so a blocked score piece would
                # otherwise head-of-line-block everything behind it.
                pending = []

                def filler(n=1):
                    for _ in range(n):
                        if not pending:
                            return
                        pending.pop(0)()

                # Scores psum pieces are PIECE_W packed columns wide and span
                # k-tile boundaries, so one exp instruction covers ~1536
                # columns — a third of the per-instruction ACT overhead of
                # the per-tile layout.
                PIECE_W = 1024
                sstt = {"cur": None, "p": -1, "tris": [], "deferred": []}

                def flush_piece(h):
                    if sstt["cur"] is None:
                        return
                    ep = eps[h % 3]
                    p = sstt["p"]
                    base = PIECE_W * p
                    wid = min(PIECE_W, EXP_TOT - base)
                    nc.scalar.activation(
                        out=ep[:, base:base + wid],
                        in_=sstt["cur"][:, 0:wid],
                        func=mybir.ActivationFunctionType.Exp,
                        scale=0.125,
                    )
                    # triangular masks for diagonal blocks inside this piece
                    for toff in sstt["tris"]:
                        nc.vector.tensor_tensor(
                            out=ep[:, toff:toff + P], in0=ep[:, toff:toff + P],
                            in1=tri_sb[:], op=mybir.AluOpType.mult,
                        )
                    sstt["tris"] = []
                    sstt["cur"] = None
                    # release work that was waiting for masks up to this col
                    done = base + wid
                    rest = []
                    for thr, fn in sstt["deferred"]:
                        if thr <= done:
                            fn()
                        else:
                            rest.append((thr, fn))
                    sstt["deferred"] = rest
                    filler(3)

                def emit_scores(h, j):
                    th, bs = h // 2, 64 * (h % 2)
                    qh = qrot[bs:bs + 64, th, :]
                    kh = krot[bs:bs + 64, th, :]
                    c0, r = j // 4, j % 4
                    off = int(_OFF[j])
                    for c in range(c0, 4):
                        q0 = 512 * c + (128 * r if c == c0 else 0)
                        q1 = 512 * (c + 1)
                        a = off + q0 - 128 * j
                        while q0 < q1:
                            p = a // PIECE_W
                            if p != sstt["p"] or sstt["cur"] is None:
                                flush_piece(h)
                                sstt["p"] = p
                                tag = ("sA", "sB")[piece_ctr[0] % 2]
                                piece_ctr[0] += 1
                                sstt["cur"] = psum.tile(
                                    [P, PIECE_W], F32, tag=tag, bufs=1,
                                    name="sp",
                                )
                            # split at 512-col (psum bank / zero-region)
                            # boundaries; the packed fill is gapless left to
                            # right, so a write starting at a bank boundary
                            # opens that bank's accumulation group and a
                            # write ending at one closes it.
                            take = min(q1 - q0, 512 - a % 512)
                            nc.tensor.matmul(
                                sstt["cur"][:, a - PIECE_W * p:
                                            a - PIECE_W * p + take],
                                kh[:, P * j:P * (j + 1)],
                                qh[:, q0:q0 + take],
                                start=(a % 512 == 0),
                                stop=((a + take) % 512 == 0),
                                skip_group_check=True,
                            )
                            q0 += take
                            a += take
                    sstt["tris"].append(off)

                def queue_pv(h, c):
                    ep = eps[h % 3]
                    th, bs = h // 2, 64 * (h % 2)
                    state = {}
                    last_j = 4 * c + 3

                    def mk_slice(j0):
                        def emit():
                            if "pv" not in state:
                                state["pv"] = psum.tile(
                                    [P, 512], F32, tag="proj", bufs=3,
                                    name="pv",
                                )
                            pv = state["pv"]
                            for j in range(j0, min(j0 + 3, last_j + 1)):
                                off = int(_OFF[j])
                                if j // 4 == c:
                                    rr = j % 4
                                    n = 512 - 128 * rr
                                    nc.tensor.matmul(
                                        pv[0:DH + 1, 128 * rr:512],
                                        v_sb[:, j, h, :],
                                        ep[:, off:off + n],
                                        start=(j == 0), stop=(j == last_j),
                                    )
                                else:
                                    st_col = off + 512 * c - 128 * j
                                    nc.tensor.matmul(
                                        pv[0:DH + 1, :],
                                        v_sb[:, j, h, :],
                                        ep[:, st_col:st_col + 512],
                                        start=(j == 0), stop=(j == last_j),
                                    )
                        return emit

                    def finalize():
                        pv = state["pv"]
                        den = atmp.tile([1, 512], F32, tag="den")
                        nc.vector.tensor_copy(out=den[:], in_=pv[DH:DH + 1, :])
                        recip = atmp.tile([1, 512], F32, tag="recip")
                        nc.vector.reciprocal_approx_fast(
                            out=recip[:], in_=den[:]
                        )
                        bcast = atmp.tile([64, 512], F32, tag="bcast")
                        nc.gpsimd.partition_broadcast(bcast[:], recip[:])
                        nc.vector.tensor_tensor(
                            out=at_sb[bs:bs + 64, th, 512 * c:512 * (c + 1)],
                            in0=pv[0:DH, :], in1=bcast[:],
                            op=mybir.AluOpType.mult,
                        )

                    for j0 in range(0, last_j + 1, 3):
                        pending.append(mk_slice(j0))
                    pending.append(finalize)

                def queue_outproj(sc):
                    # outT_partial[:, sc] = sum over the 256 LOCAL attention
                    # dims (this core's 4 heads); host sums the partials
                    ssl = bass.ts(sc, 512)

                    def mk_block(ot):
                        def emit():
                            osl = bass.ts(ot, P)
                            if ot % 2 == 0:
                                po = psum.tile(
                                    [P, 512], F32, tag="proj", bufs=3,
                                    name="po",
                                )
                            else:
                                po = psum.tile(
                                    [P, 512], F32, tag="shuf", bufs=1,
                                    name="po",
                                )
                            for ct in range(2):
                                nc.tensor.matmul(
                                    po[:, 0:512],
                                    wo_sb[:, ct, osl],
                                    at_sb[:, ct, ssl],
                                    start=(ct == 0), stop=(ct == 1),
                                )
                            ob = agp.tile([P, 512], F16, tag="ob", name="ob")
                            if ot % 2 == 0:
                                nc.vector.tensor_copy(
                                    out=ob[:], in_=po[:, 0:512]
                                )
                            else:
                                nc.scalar.activation(
                                    out=ob[:], in_=po[:, 0:512],
                                    func=mybir.ActivationFunctionType.Copy,
                                    scale=1.0,
                                )
                            oq = (nc.sync, nc.gpsimd, nc.scalar)[ot % 3]
                            oq.dma_start(
                                out=outT[P * ot:P * (ot + 1), ssl], in_=ob[:]
                            )
                        return emit

                    for ot in range(8):
                        pending.append(mk_block(ot))

                # software pipeline: score pieces of head h clock the stream;
                # after each piece one pending item (a PV slice of head h-1,
                # a softmax finalize, or an outproj block) is emitted so the
                # PE always has ready work directly behind a piece that still
                # waits on exp.
                for h in range(HPG):
                    for j in range(NKT):
                        emit_scores(h, j)
                        if h >= 1 and j % 4 == 1:
                            queue_pv(h - 1, j // 4)
                        if h == 3 and j % 4 == 3:
                            c = j // 4
                            if c < 3:
                                # head-3 PV must not enter the stream until
                                # the tri-mask of its last tile has been
                                # emitted (at the flush covering that block)
                                thr = int(_OFF[j]) + P

                                def mk_pv(cc):
                                    def fn():
                                        queue_pv(3, cc)
                                    return fn

                                def mk_op(cc):
                                    def fn():
                                        queue_outproj(cc)
                                    return fn

                                sstt["deferred"].append((thr, mk_pv(c)))
                                sstt["deferred"].append(
                                    (min(thr + 3072, EXP_TOT), mk_op(c))
                                )
                    flush_piece(h)
                queue_pv(3, 3)
                queue_outproj(3)
                while pending:
                    pending.pop(0)()

    nc.compile()
    return nc


_PROGRAM = None


def _get_program():
    global _PROGRAM
    if _PROGRAM is None:
        _PROGRAM = build_program()
    return _PROGRAM


def _host_consts(token_positions):
    pos = np.asarray(token_positions, dtype=np.float32)
    inv = (
        ROPE_THETA ** (-np.arange(0, DH, 2, dtype=np.float32) / DH)
    ).astype(np.float32)
    ang = pos[:, None] * inv[None, :]  # [S, 32]
    cos, sin = np.cos(ang), np.sin(ang)
    rows = (np.arange(P) % DH) // 2
    cosT = np.ascontiguousarray(cos.T[rows]).astype(np.float16)
    sinT = np.ascontiguousarray(sin.T[rows]).astype(np.float16)
    Smat = np.zeros((P, P), dtype=np.float32)
    idx = np.arange(0, P, 2)
    Smat[idx, idx + 1] = -1.0
    Smat[idx + 1, idx] = 1.0
    ST = np.ascontiguousarray(Smat.T).astype(np.float16)
    tri = (np.arange(P)[None, :] >= np.arange(P)[:, None]).astype(np.float16)
    return cosT, sinT, ST, tri


def _make_in_maps(x, W_q, W_k, W_v, W_o, token_positions):
    cosT, sinT, ST, tri = _host_consts(token_positions)
    x = np.asarray(x, dtype=np.float32)
    maps = []
    for core in range(NCORE):
        b, hg = core // 4, core % 4
        hsl = slice(256 * hg, 256 * (hg + 1))
        # W_o columns for this core's local attention dims (its 4 heads);
        # each core emits a full [1024, 2048] partial that the host sums.
        wo_p = np.asarray(W_o, dtype=np.float32)[:, hsl].T   # [256 c, 1024 o]
        maps.append(
            {
                "xT": np.ascontiguousarray(x[b].T).astype(np.float16),
                "wqT": np.ascontiguousarray(np.asarray(W_q, np.float32)[hsl].T).astype(np.float16),
                "wkT": np.ascontiguousarray(np.asarray(W_k, np.float32)[hsl].T).astype(np.float16),
                "wvT": np.ascontiguousarray(np.asarray(W_v, np.float32)[hsl].T).astype(np.float16),
                "woT": np.ascontiguousarray(wo_p).astype(np.float16),
                "cosT": cosT,
                "sinT": sinT,
                "ST": ST,
                "trimask": tri,
            }
        )
    return maps


def _assemble(results):
    out = np.zeros((B, S, D), dtype=np.float32)
    for core in range(NCORE):
        b = core // 4
        out[b] += results[core]["outT"].astype(np.float32).T
    return out


def _run(in_maps, trace=False):
    nc = _get_program()
    tmpdir = None
    if trace:
        import tempfile

        tmpdir = tempfile.mkdtemp(prefix="ntff_", dir="/tmp")
    res = run_bass_kernel_spmd(
        nc, in_maps, list(range(NCORE)), trace=trace, tmpdir=tmpdir
    )
    return res


def kernel(x, W_q, W_k, W_v, W_o, token_positions):
    in_maps = _make_in_maps(x, W_q, W_k, W_v, W_o, token_positions)
    res = _run(in_maps)
    return _assemble(res.results)


def _install_profile_hook():
    """The agent image's antenv lacks axon_hooks; shim it so trace=True works."""
    import sys
    import types

    try:
        from antenv.axon_hooks import get_axon_ntff_profile_hook  # noqa: F401
        return
    except ImportError:
        pass
    import antenv
    from trn_agent_boot.trn_boot import _ntff_profile_via_ctypes

    mod = types.ModuleType("antenv.axon_hooks")
    _hook = {"h": None}
    mod.set_axon_ntff_profile_hook = lambda h: _hook.__setitem__("h", h)
    mod.get_axon_ntff_profile_hook = lambda: _hook["h"]
    sys.modules["antenv.axon_hooks"] = mod
    antenv.axon_hooks = mod
    mod.set_axon_ntff_profile_hook(
        _ntff_profile_via_ctypes("/opt/axon/libaxon_pjrt.so")
    )
    import concourse.bass_utils as bu

    bu.upload_artifacts = lambda d: f"file://{d}"


def kernel_traced(x, W_q, W_k, W_v, W_o, token_positions):
    """Returns (output, exec_time_ns, trace_path)."""
    _install_profile_hook()
    in_maps = _make_in_maps(x, W_q, W_k, W_v, W_o, token_positions)
    res = _run(in_maps, trace=True)
    trace_path = None
    if res.instructions_and_trace is not None:
        trace_path = res.instructions_and_trace[1]
    return _assemble(res.results), res.exec_time_ns, trace_path

